# revision 7
# baseline (speedup 1.0000x reference)
"""AUGRU (VecAttGRUCell) dynamic_rnn kernel for Trainium2, 8 NeuronCores.

Problem: B=1024, T=512, D=128 (fp32).
    gi = [x, h] @ gate_kernel + gate_bias ; r, u = split(sigmoid(gi))
    c  = tanh([x, r*h] @ cand_kernel + cand_bias)
    u' = (1 - att) * u ; h' = u'*h + (1-u')*c
    out[t] = h' for t < len, else 0 ; h frozen past len.

Sharding: batch 1024 -> 8 cores x 128 rows, scan over T local per core,
weights replicated.

Wall time in this environment is dominated by the host<->device
transport (~40 MB/s tunnel), so the design minimizes bytes moved and
host work:

* X ships in its natural [B, T, D] layout as fp16 (half the bytes; the
  AUGRU is contractive enough that fp16 input+output quantization costs
  ~1.2e-3 relative error vs the 2e-2 gate). Per-core shards are
  converted fp32->fp16 one at a time and device_put asynchronously, so
  conversion overlaps the wire transfer.
* The PE transposes each x_t on-chip ([BSH, D] -> [D, BSH] fp16 via
  identity matmul) into a PSUM scratch bank; the recurrence runs
  feature-major in fp32. Each h' is PE-transposed back and masked on the
  ACT engine (Copy activation with a per-partition (t < len) scale), so
  the output leaves the device already masked, in natural [BSH, T, D]
  fp16 layout. The host does no transposes, no concat, no masking.
* Output shards are fetched with copy_to_host_async and upcast to fp32
  into the result buffer while later shards are still in flight.
* The executor bypasses run_bass_kernel_spmd when running under axon: a
  jit(shard_map(bass_exec)) callable is built once and cached, inputs
  are globals whose axis-0 shards are exactly the per-core shapes, and
  the donated ExternalOutput zero-buffer is created on-device. On a
  native (non-axon) runtime it falls back to run_bass_kernel_spmd with
  the same BIR.

Per step the serial h -> h' chain (~7 engine hops):
  whr MM -> sigma_r (ACT, bias AP) -> rh (DVE) -> ch MM -> tanh (ACT)
  -> g = (z-1)*c (DVE STT) -> h' = p - g (DVE), with the u-path
  (whu MM, sigma_u, z = u*alpha_bcast, p = z*h on GPSIMD) off-chain.
x-projections (fp16 weights) and the rank-1 alpha broadcast are batched
4 steps per matmul; the output transpose+mask for step i is emitted
during step i+1 so it lands in PE/ACT idle windows. PSUM: pr/pu (2x2) +
pc (2) + alpha (1) + shared xT/hT scratch (1) = 8 banks.

Measured on the staged test harness (t=512, wall per call, best of 3):
27.4 s for the previous version -> 6.7 s for this one; device exec
itself is ~90 ms. Relative error 1.23e-3.
"""

import numpy as np

import concourse.bacc as bacc
import concourse.mybir as mybir
import concourse.tile as tile
import concourse.bass as bass

F32 = mybir.dt.float32
F16 = mybir.dt.float16
AF = mybir.ActivationFunctionType
OP = mybir.AluOpType

B, T, D = 1024, 512, 128
NCORES = 8
BSH = B // NCORES          # batch rows per core = 128
CHUNK = 32                 # timesteps per DMA chunk

_runner_cache = {}
_nc_cache = {}


def _emit_chunk(nc, pools, consts, h_cur, c0, xch, ach, mch, OUT, chunk,
                dyn=False, tag=""):
    """Emit one chunk (`chunk` timesteps) starting at step c0 (int when
    unrolled, RuntimeValue under For_i). Returns the AP holding the final
    h."""
    wpool, xtpool, hopool, pru_pool, pc_pool, pa_pool, scr_pool = pools
    (xw16, whr, whu, ch, gbr, gbu, cbc, ones, idt, idt16) = consts

    for q in range(chunk // 4):
        q0 = q * 4
        # transpose 4 x_t's: [BSH, D] -> [D, BSH] via PE (fp16), stage in SBUF
        xt_ps = scr_pool.tile([128, 4, 128], F16, tag="scr",
                              padded_shape=[128, 4, 256],
                              name=f"xtp_{tag}_{q}")
        for i in range(4):
            nc.tensor.transpose(xt_ps[:, i, :], xch[:, q0 + i, :], idt16[:])
        xt4 = xtpool.tile([D, 4, BSH], F16, tag="xt", name=f"xt_{tag}_{q}")
        nc.scalar.activation(xt4[:], xt_ps[:], AF.Copy)

        pr4 = pru_pool.tile([D, 4, BSH], F32, tag="pr4", name=f"pr4_{tag}_{q}")
        pu4 = pru_pool.tile([D, 4, BSH], F32, tag="pu4", name=f"pu4_{tag}_{q}")
        pc4 = pc_pool.tile([D, 4, BSH], F32, tag="pc4", name=f"pc4_{tag}_{q}")
        pa4 = pa_pool.tile([D, 4, BSH], F32, tag="pa4", name=f"pa4_{tag}_{q}")
        nc.tensor.matmul(pr4[:], xw16[:, 0, :], xt4[:], start=True, stop=True)
        nc.tensor.matmul(pu4[:], xw16[:, 1, :], xt4[:], start=True, stop=True)
        nc.tensor.matmul(pc4[:], xw16[:, 2, :], xt4[:], start=True, stop=True)
        nc.tensor.matmul(pa4[:], ones[:], ach[0:1, bass.ts(q, 4 * BSH)],
                         start=True, stop=True)

        ht_ps = scr_pool.tile([128, 4, 128], F32, tag="scr",
                              name=f"htp_{tag}_{q}")
        ho4 = hopool.tile([BSH, 4, D], F16, tag="ho", name=f"ho_{tag}_{q}")

        def emit_out(j, h_j):
            # output path for step j: PE transpose back to [BSH, D], then
            # mask on ACT (Copy with per-partition scale m_t); emitted one
            # step late so it lands in PE/ACT idle windows off the chain
            nc.tensor.transpose(ht_ps[:, j, :], h_j, idt[:])
            nc.scalar.activation(ho4[:, j, :], ht_ps[:, j, :], AF.Copy,
                                 scale=mch[:, q0 + j : q0 + j + 1])

        for i in range(4):
            h_c = h_cur
            # --- critical chain ---------------------------------------
            nc.tensor.matmul(pr4[:, i, :], whr[:], h_c,
                             start=False, stop=True, skip_group_check=True)
            r_t = wpool.tile([D, BSH], F32, tag="r", name=f"r_{tag}_{q}_{i}")
            nc.scalar.activation(r_t[:], pr4[:, i, :], AF.Sigmoid, bias=gbr[:])
            # u-path interleaved so in-order ACT does sigma_u in the gap
            nc.tensor.matmul(pu4[:, i, :], whu[:], h_c,
                             start=False, stop=True, skip_group_check=True)
            u_t = wpool.tile([D, BSH], F32, tag="u", name=f"u_{tag}_{q}_{i}")
            nc.scalar.activation(u_t[:], pu4[:, i, :], AF.Sigmoid, bias=gbu[:])
            rh = wpool.tile([D, BSH], F32, tag="rh", name=f"rh_{tag}_{q}_{i}")
            nc.vector.tensor_mul(rh[:], r_t[:], h_c)
            nc.tensor.matmul(pc4[:, i, :], ch[:], rh[:],
                             start=False, stop=True, skip_group_check=True)
            c_t = wpool.tile([D, BSH], F32, tag="c", name=f"c_{tag}_{q}_{i}")
            nc.scalar.activation(c_t[:], pc4[:, i, :], AF.Tanh, bias=cbc[:])
            # --- off-chain tail ---------------------------------------
            z = wpool.tile([D, BSH], F32, tag="z", name=f"z_{tag}_{q}_{i}")
            nc.vector.tensor_mul(z[:], u_t[:], pa4[:, i, :])
            p_t = wpool.tile([D, BSH], F32, tag="p", name=f"p_{tag}_{q}_{i}")
            nc.gpsimd.tensor_mul(p_t[:], z[:], h_c)
            # h' = z*h + (1-z)*c = p - (z-1)*c
            g_t = wpool.tile([D, BSH], F32, tag="g", name=f"g_{tag}_{q}_{i}")
            nc.vector.scalar_tensor_tensor(g_t[:], z[:], 1.0, c_t[:],
                                           OP.subtract, OP.mult)
            h_new = wpool.tile([D, BSH], F32, tag="h", name=f"h_{tag}_{q}_{i}")
            nc.vector.tensor_sub(h_new[:], p_t[:], g_t[:])
            if i > 0:
                emit_out(i - 1, h_prev)
            h_prev = h_new[:]
            h_cur = h_new[:]
        emit_out(3, h_prev)
        if dyn:
            nc.sync.dma_start(OUT[:, bass.ds(c0 + q0, 4), :], ho4[:])
        else:
            nc.sync.dma_start(OUT[:, c0 + q0 : c0 + q0 + 4, :], ho4[:])
    return h_cur


def _build(nc, t_steps, chunk, looped, h_io=False):
    nchunks = t_steps // chunk
    X = nc.dram_tensor("X", (BSH, t_steps, D), F16, kind="ExternalInput")
    A = nc.dram_tensor("A", (1, t_steps * BSH), F32, kind="ExternalInput")
    M = nc.dram_tensor("M", (BSH, t_steps), F32, kind="ExternalInput")
    GK = nc.dram_tensor("GK", (2 * D, 2 * D), F32, kind="ExternalInput")
    CK = nc.dram_tensor("CK", (2 * D, D), F32, kind="ExternalInput")
    GBR = nc.dram_tensor("GBR", (D, 1), F32, kind="ExternalInput")
    GBU = nc.dram_tensor("GBU", (D, 1), F32, kind="ExternalInput")
    CBC = nc.dram_tensor("CBC", (D, 1), F32, kind="ExternalInput")
    IDT = nc.dram_tensor("IDT", (128, 128), F32, kind="ExternalInput")
    XW = nc.dram_tensor("XW", (D, 3 * D), F16, kind="ExternalInput")
    if h_io:
        HIN = nc.dram_tensor("HIN", (D, BSH), F32, kind="ExternalInput")
        HOUT = nc.dram_tensor("HOUT", (D, BSH), F32,
                              kind="ExternalOutput")
    OUT = nc.dram_tensor("OUT", (BSH, t_steps, D), F16, kind="ExternalOutput")

    with tile.TileContext(nc) as tc:
        with (
            tc.tile_pool(name="const", bufs=1) as constp,
            tc.tile_pool(name="xch", bufs=2) as xpool,
            tc.tile_pool(name="ach", bufs=2) as apool,
            tc.tile_pool(name="mch", bufs=2) as mpool,
            tc.tile_pool(name="xt", bufs=2) as xtpool,
            tc.tile_pool(name="work", bufs=3) as wpool,
            tc.tile_pool(name="ho", bufs=2) as hopool,
            tc.tile_pool(name="pru", bufs=2, space="PSUM") as pru_pool,
            tc.tile_pool(name="pc", bufs=2, space="PSUM") as pc_pool,
            tc.tile_pool(name="pa", bufs=1, space="PSUM") as pa_pool,
            tc.tile_pool(name="scr", bufs=1, space="PSUM") as scr_pool,
        ):
            pools = (wpool, xtpool, hopool, pru_pool, pc_pool, pa_pool,
                     scr_pool)
            xw16 = constp.tile([D, 3, D], F16, tag="xw16")
            whr = constp.tile([D, D], F32, tag="whr")
            whu = constp.tile([D, D], F32, tag="whu")
            ch = constp.tile([D, D], F32, tag="ch")
            gbr = constp.tile([D, 1], F32, tag="gbr")
            gbu = constp.tile([D, 1], F32, tag="gbu")
            cbc = constp.tile([D, 1], F32, tag="cbc")
            ones = constp.tile([1, D], F32, tag="ones")
            idt = constp.tile([128, 128], F32, tag="idt")
            idt16 = constp.tile([128, 128], F16, tag="idt16")
            consts = (xw16, whr, whu, ch, gbr, gbu, cbc, ones, idt, idt16)

            nc.sync.dma_start(xw16[:], XW[:])
            nc.sync.dma_start(whr[:], GK[D : 2 * D, 0:D])
            nc.sync.dma_start(whu[:], GK[D : 2 * D, D : 2 * D])
            nc.sync.dma_start(ch[:], CK[D : 2 * D, :])
            nc.sync.dma_start(gbr[:], GBR[:])
            nc.sync.dma_start(gbu[:], GBU[:])
            nc.sync.dma_start(cbc[:], CBC[:])
            nc.sync.dma_start(idt[:], IDT[:])
            nc.scalar.activation(idt16[:], idt[:], AF.Copy)
            nc.gpsimd.memset(ones[:], 1.0)

            hst = constp.tile([D, BSH], F32, tag="hst", name="h_state")
            if h_io:
                nc.sync.dma_start(hst[:], HIN[:])
            else:
                nc.gpsimd.memset(hst[:], 0.0)
            if looped:
                # fixed-address state tile: each iteration starts and ends
                # with h in hst
                with tc.For_i(0, nchunks, 1) as ci:
                    c0 = ci * chunk
                    xch = xpool.tile([BSH, chunk, D], F16, tag="xch",
                                     name="xch")
                    nc.sync.dma_start(xch[:], X[:, bass.ds(c0, chunk), :])
                    ach = apool.tile([1, chunk * BSH], F32, tag="ach",
                                     name="ach")
                    nc.sync.dma_start(
                        ach[:], A[0:1, bass.ds(c0 * BSH, chunk * BSH)])
                    mch = mpool.tile([BSH, chunk], F32, tag="mch",
                                     name="mch")
                    nc.sync.dma_start(mch[:], M[:, bass.ds(c0, chunk)])
                    h_end = _emit_chunk(nc, pools, consts, hst[:], c0,
                                        xch, ach, mch, OUT, chunk,
                                        dyn=True, tag="L")
                    nc.vector.tensor_copy(hst[:], h_end)
                if h_io:
                    nc.sync.dma_start(HOUT[:], hst[:])
            else:
                h_cur = hst[:]
                for ci in range(nchunks):
                    c0 = ci * chunk
                    xch = xpool.tile([BSH, chunk, D], F16, tag="xch",
                                     name=f"xch_{ci}")
                    nc.sync.dma_start(xch[:], X[:, c0 : c0 + chunk, :])
                    ach = apool.tile([1, chunk * BSH], F32, tag="ach",
                                     name=f"ach_{ci}")
                    nc.sync.dma_start(ach[:],
                                      A[0:1, c0 * BSH : (c0 + chunk) * BSH])
                    mch = mpool.tile([BSH, chunk], F32, tag="mch",
                                     name=f"mch_{ci}")
                    nc.sync.dma_start(mch[:], M[:, c0 : c0 + chunk])
                    h_cur = _emit_chunk(nc, pools, consts, h_cur, c0,
                                        xch, ach, mch, OUT, chunk,
                                        dyn=False, tag=str(ci))

    nc.finalize()
    return nc


def _get_runner(t_steps, looped, h_io=False, n_cores=NCORES):
    key = (t_steps, looped, h_io, n_cores)
    if key in _runner_cache:
        return _runner_cache[key]

    import jax
    import jax.numpy as jnp
    from jax.sharding import Mesh, NamedSharding, PartitionSpec
    from jax.experimental.shard_map import shard_map
    from concourse.bass2jax import (
        _bass_exec_p,
        install_neuronx_cc_hook,
        partition_id_tensor,
    )

    install_neuronx_cc_hook()

    bkey = (t_steps, looped, h_io)
    nc = _nc_cache.get(bkey)
    if nc is None:
        nc = bacc.Bacc("TRN2", target_bir_lowering=False)
        nc = _build(nc, t_steps, CHUNK, looped, h_io)
        _nc_cache[bkey] = nc
    assert nc.dbg_addr is None
    partition_name = (
        nc.partition_id_tensor.name if nc.partition_id_tensor else None
    )

    in_names, out_names, out_avals = [], [], []
    for alloc in nc.m.functions[0].allocations:
        if not isinstance(alloc, mybir.MemoryLocationSet):
            continue
        name = alloc.memorylocations[0].name
        if alloc.kind == "ExternalInput":
            if name != partition_name:
                in_names.append(name)
        elif alloc.kind == "ExternalOutput":
            assert alloc.tensor_shape is not None and alloc.dtype is not None
            out_names.append(name)
            out_avals.append(
                jax.core.ShapedArray(
                    tuple(alloc.tensor_shape), mybir.dt.np(alloc.dtype)
                )
            )
    n_params = len(in_names)
    n_outs = len(out_names)
    all_in_names = tuple(in_names) + tuple(out_names)
    if partition_name is not None:
        all_in_names = all_in_names + (partition_name,)

    devices = jax.devices()[:n_cores]
    assert len(devices) == n_cores
    mesh = Mesh(np.asarray(devices), ("core",))

    def _body(*args):
        operands = list(args)
        if partition_name is not None:
            operands.append(partition_id_tensor())
        outs = _bass_exec_p.bind(
            *operands,
            out_avals=tuple(out_avals),
            in_names=all_in_names,
            out_names=tuple(out_names),
            lowering_input_output_aliases=(),
            sim_require_finite=True,
            sim_require_nnan=True,
            nc=nc,
        )
        return tuple(outs)

    donate = tuple(range(n_params, n_params + n_outs))
    sharded = jax.jit(
        shard_map(
            _body,
            mesh=mesh,
            in_specs=(PartitionSpec("core"),) * (n_params + n_outs),
            out_specs=(PartitionSpec("core"),) * n_outs,
            check_rep=False,
        ),
        donate_argnums=donate,
        keep_unused=True,
    )

    out_sharding = NamedSharding(mesh, PartitionSpec("core"))
    zspecs = tuple(
        ((n_cores * a.shape[0],) + tuple(a.shape[1:]), jnp.dtype(a.dtype))
        for a in out_avals
    )
    zeros_fn = jax.jit(
        lambda: tuple(jnp.zeros(s, d) for s, d in zspecs),
        out_shardings=(out_sharding,) * n_outs,
    )

    runner = (sharded, zeros_fn, list(in_names), list(out_names),
              list(devices), out_sharding)
    _runner_cache[key] = runner
    return runner


TH = 256                   # stage split point on the time axis
K1 = 5                     # cores for stage 1 (640 rows with len > TH)
B1 = K1 * BSH


def _weight_arrays(gk, gb, ck, cb, n_cores):
    return {
        "GK": np.tile(gk, (n_cores, 1)),
        "CK": np.tile(ck, (n_cores, 1)),
        "GBR": np.tile(gb[:D].reshape(D, 1), (n_cores, 1)),
        "GBU": np.tile(gb[D:].reshape(D, 1), (n_cores, 1)),
        "CBC": np.tile(cb.reshape(D, 1), (n_cores, 1)),
        "IDT": np.tile(np.eye(128, dtype=np.float32), (n_cores, 1)),
        "XW": np.tile(
            np.concatenate([gk[:D, :D], gk[:D, D:], ck[:D, :]], axis=1)
            .astype(np.float16),
            (n_cores, 1),
        ),
    }


def _fetch_shards(garr):
    shards = sorted(garr.addressable_shards, key=lambda s: s.index[0].start)
    for s in shards:
        try:
            s.data.copy_to_host_async()
        except Exception:
            pass
    return shards


def _run_staged(jax, rnn_input, att, gk, gb, ck, cb, lens):
    """Length-split execution: rows past their sequence length are dead
    data, so stage 0 runs t<TH for all rows and stage 1 runs t>=TH for
    only the (padded-to-B1) rows with len>TH, cutting wire bytes ~19%.
    Returns None when the length distribution doesn't fit (caller falls
    back to the single-call path)."""
    idx = np.nonzero(lens > TH)[0]
    K = idx.size
    if K > B1:
        return None

    r0 = _get_runner(TH, True, True, NCORES)
    r1 = _get_runner(TH, True, True, K1)
    sharded0, zfn0, in_names0, out_names0, devs0, shrd0 = r0
    sharded1, zfn1, in_names1, out_names1, devs1, shrd1 = r1
    z0 = zfn0()

    xsrc = np.asarray(rnn_input)
    alpha = 1.0 - att[:, :, 0]                               # [B, T]
    mask = (
        np.arange(T, dtype=np.int32)[None, :] < lens[:, None]
    ).astype(np.float32)                                      # [B, T]

    # ---- stage 0: all rows, t in [0, TH) ----
    xshards0 = [
        jax.device_put(
            np.ascontiguousarray(xsrc[c * BSH : (c + 1) * BSH, :TH])
            .astype(np.float16),
            devs0[c],
        )
        for c in range(NCORES)
    ]
    Xg0 = jax.make_array_from_single_device_arrays(
        (B, TH, D), shrd0, xshards0
    )
    arrays0 = _weight_arrays(gk, gb, ck, cb, NCORES)
    arrays0["X"] = Xg0
    arrays0["A"] = np.ascontiguousarray(
        alpha[:, :TH].reshape(NCORES, BSH, TH).transpose(0, 2, 1)
    ).reshape(NCORES, TH * BSH)
    arrays0["M"] = np.ascontiguousarray(mask[:, :TH])
    arrays0["HIN"] = np.zeros((NCORES * D, BSH), np.float32)
    outs0 = dict(zip(out_names0, sharded0(
        *[arrays0[n] for n in in_names0], *z0)))

    # ---- stage 1 host prep (overlaps stage-0 transfers/exec) ----
    rows = np.zeros(B1, np.int64)
    rows[:K] = idx
    xshards1 = [
        jax.device_put(
            xsrc[rows[c * BSH : (c + 1) * BSH], TH:].astype(np.float16),
            devs1[c],
        )
        for c in range(K1)
    ]
    Xg1 = jax.make_array_from_single_device_arrays(
        (B1, TH, D), shrd1, xshards1
    )
    arrays1 = _weight_arrays(gk, gb, ck, cb, K1)
    arrays1["X"] = Xg1
    arrays1["A"] = np.ascontiguousarray(
        alpha[rows, TH:].reshape(K1, BSH, TH).transpose(0, 2, 1)
    ).reshape(K1, TH * BSH)
    m1 = mask[rows, TH:]
    m1[K:] = 0.0                      # padding rows fully masked
    arrays1["M"] = np.ascontiguousarray(m1)

    # h(TH) for the stage-1 rows (blocks on stage-0 completion)
    h0 = np.asarray(outs0["HOUT"]).reshape(NCORES, D, BSH)
    hvec = h0[rows // BSH, :, rows % BSH]                    # [B1, D]
    arrays1["HIN"] = np.ascontiguousarray(
        hvec.reshape(K1, BSH, D).transpose(0, 2, 1)
    ).reshape(K1 * D, BSH)

    # start OUT0 d2h before dispatching stage 1
    shards0 = _fetch_shards(outs0["OUT"])
    z1 = zfn1()
    outs1 = dict(zip(out_names1, sharded1(
        *[arrays1[n] for n in in_names1], *z1)))

    # ---- assemble: calloc keeps the (len<=TH, t>=TH) tails zero ----
    res = np.zeros((B, T, D), np.float32)
    for s in shards0:
        res[s.index[0], :TH] = np.asarray(s.data)
    shards1 = _fetch_shards(outs1["OUT"])
    o1 = np.empty((B1, TH, D), np.float32)
    for s in shards1:
        o1[s.index[0]] = np.asarray(s.data)
    res[idx, TH:] = o1[:K]
    return res


def kernel(rnn_input, att_score, gate_kernel, gate_bias, cand_kernel,
           cand_bias, sequence_length, _t_steps: int = T,
           _looped: bool = True):
    """Full-input entry point: shard across 8 cores, run, gather."""
    t_steps = _t_steps
    att = np.asarray(att_score, dtype=np.float32)
    gk = np.ascontiguousarray(np.asarray(gate_kernel, dtype=np.float32))
    gb = np.asarray(gate_bias, dtype=np.float32).reshape(2 * D)
    ck = np.ascontiguousarray(np.asarray(cand_kernel, dtype=np.float32))
    cb = np.asarray(cand_bias, dtype=np.float32).reshape(D)
    lens = np.asarray(sequence_length, dtype=np.int32).reshape(-1)

    from concourse._compat import axon_active

    fast = axon_active()
    if fast and t_steps == T and _looped:
        import jax

        res = _run_staged(jax, rnn_input, att, gk, gb, ck, cb, lens)
        if res is not None:
            return res
    if fast:
        import jax

        sharded, zeros_fn, in_names, out_names, devices, out_sharding = (
            _get_runner(t_steps, _looped)
        )
        zeros = zeros_fn()                   # async device-side zero-fill

        # per-shard h2d of X: convert shard c+1 to fp16 while shard c is
        # in flight over the transport
        xsrc = np.asarray(rnn_input)[:, :t_steps, :]
        xshards = [
            jax.device_put(
                np.ascontiguousarray(xsrc[c * BSH : (c + 1) * BSH])
                .astype(np.float16),
                devices[c],
            )
            for c in range(NCORES)
        ]
        Xg = jax.make_array_from_single_device_arrays(
            (B, t_steps, D), out_sharding, xshards
        )
    else:
        Xg = np.asarray(rnn_input)[:, :t_steps, :].astype(np.float16)

    alpha = 1.0 - att[:, :t_steps, 0]                       # [B, t]
    A = np.ascontiguousarray(
        alpha.reshape(NCORES, BSH, t_steps).transpose(0, 2, 1)
    ).reshape(NCORES, t_steps * BSH)
    M = (
        np.arange(t_steps, dtype=np.int32)[None, :]
        < np.minimum(lens, t_steps)[:, None]
    ).astype(np.float32)                                     # [B, t]

    arrays = {
        "X": Xg,
        "A": A,
        "M": M,
        "GK": np.tile(gk, (NCORES, 1)),
        "CK": np.tile(ck, (NCORES, 1)),
        "GBR": np.tile(gb[:D].reshape(D, 1), (NCORES, 1)),
        "GBU": np.tile(gb[D:].reshape(D, 1), (NCORES, 1)),
        "CBC": np.tile(cb.reshape(D, 1), (NCORES, 1)),
        "IDT": np.tile(np.eye(128, dtype=np.float32), (NCORES, 1)),
        "XW": np.tile(
            np.concatenate([gk[:D, :D], gk[:D, D:], ck[:D, :]], axis=1)
            .astype(np.float16),
            (NCORES, 1),
        ),
    }
    if not fast:
        # native (non-axon) path: same BIR via the stock SPMD runner
        from concourse.bass_utils import run_bass_kernel_spmd

        nc = _nc_cache.get((t_steps, _looped))
        if nc is None:
            nc = bacc.Bacc("TRN2", target_bir_lowering=False)
            nc = _build(nc, t_steps, CHUNK, _looped)
            _nc_cache[(t_steps, _looped)] = nc
        in_maps = []
        for c in range(NCORES):
            m = {k: v for k, v in arrays.items() if k not in ("X", "A", "M")}
            m["X"] = np.ascontiguousarray(Xg[c * BSH : (c + 1) * BSH])
            m["A"] = np.ascontiguousarray(arrays["A"][c : c + 1])
            m["M"] = np.ascontiguousarray(
                arrays["M"][c * BSH : (c + 1) * BSH]
            )
            for k in ("GK", "CK", "GBR", "GBU", "CBC", "IDT", "XW"):
                m[k] = np.ascontiguousarray(
                    arrays[k][: arrays[k].shape[0] // NCORES]
                )
            in_maps.append(m)
        res8 = run_bass_kernel_spmd(nc, in_maps, list(range(NCORES)))
        res = np.empty((B, t_steps, D), np.float32)
        for c in range(NCORES):
            res[c * BSH : (c + 1) * BSH] = res8.results[c]["OUT"]
        return res

    ins = [arrays[n] for n in in_names]
    outs = sharded(*ins, *zeros)

    # per-shard d2h with async prefetch: upcast shard c to fp32 while
    # shard c+1 is in flight
    out = outs[0]
    shards = sorted(out.addressable_shards, key=lambda s: s.index[0].start)
    for s in shards:
        try:
            s.data.copy_to_host_async()
        except Exception:
            pass
    res = np.empty((B, t_steps, D), np.float32)
    for s in shards:
        res[s.index[0]] = np.asarray(s.data)
    return res


# revision 8
# speedup vs baseline: 1.1622x; 1.1622x over previous
"""AUGRU (VecAttGRUCell) dynamic_rnn kernel for Trainium2, 8 NeuronCores.

Problem: B=1024, T=512, D=128 (fp32).
    gi = [x, h] @ gate_kernel + gate_bias ; r, u = split(sigmoid(gi))
    c  = tanh([x, r*h] @ cand_kernel + cand_bias)
    u' = (1 - att) * u ; h' = u'*h + (1-u')*c
    out[t] = h' for t < len, else 0 ; h frozen past len.

Sharding: batch 1024 -> 8 cores x 128 rows, scan over T local per core,
weights replicated.

Wall time in this environment is dominated by the host<->device
transport (~40 MB/s tunnel), so the design minimizes bytes moved and
host work:

* X ships in its natural [B, T, D] layout as fp16 (half the bytes; the
  AUGRU is contractive enough that fp16 input+output quantization costs
  ~1.2e-3 relative error vs the 2e-2 gate). Per-core shards are
  converted fp32->fp16 one at a time and device_put asynchronously, so
  conversion overlaps the wire transfer.
* The PE transposes each x_t on-chip ([BSH, D] -> [D, BSH] fp16 via
  identity matmul) into a PSUM scratch bank; the recurrence runs
  feature-major in fp32. Each h' is PE-transposed back and masked on the
  ACT engine (Copy activation with a per-partition (t < len) scale), so
  the output leaves the device already masked, in natural [BSH, T, D]
  fp16 layout. The host does no transposes, no concat, no masking.
* Output shards are fetched with copy_to_host_async and upcast to fp32
  into the result buffer while later shards are still in flight.
* The executor bypasses run_bass_kernel_spmd when running under axon: a
  jit(shard_map(bass_exec)) callable is built once and cached, inputs
  are globals whose axis-0 shards are exactly the per-core shapes, and
  the donated ExternalOutput zero-buffer is created on-device. On a
  native (non-axon) runtime it falls back to run_bass_kernel_spmd with
  the same BIR.

Per step the serial h -> h' chain (~7 engine hops):
  whr MM -> sigma_r (ACT, bias AP) -> rh (DVE) -> ch MM -> tanh (ACT)
  -> g = (z-1)*c (DVE STT) -> h' = p - g (DVE), with the u-path
  (whu MM, sigma_u, z = u*alpha_bcast, p = z*h on GPSIMD) off-chain.
x-projections (fp16 weights) and the rank-1 alpha broadcast are batched
4 steps per matmul; the output transpose+mask for step i is emitted
during step i+1 so it lands in PE/ACT idle windows. PSUM: pr/pu (2x2) +
pc (2) + alpha (1) + shared xT/hT scratch (1) = 8 banks.

Measured on the staged test harness (t=512, wall per call, best of 3):
27.4 s for the previous version -> 6.7 s for this one; device exec
itself is ~90 ms. Relative error 1.23e-3.
"""

import numpy as np

import concourse.bacc as bacc
import concourse.mybir as mybir
import concourse.tile as tile
import concourse.bass as bass

F32 = mybir.dt.float32
F16 = mybir.dt.float16
AF = mybir.ActivationFunctionType
OP = mybir.AluOpType

B, T, D = 1024, 512, 128
NCORES = 8
BSH = B // NCORES          # batch rows per core = 128
CHUNK = 32                 # timesteps per DMA chunk

_runner_cache = {}
_nc_cache = {}


def _emit_chunk(nc, pools, consts, h_cur, c0, xch, ach, mch, OUT, chunk,
                dyn=False, tag=""):
    """Emit one chunk (`chunk` timesteps) starting at step c0 (int when
    unrolled, RuntimeValue under For_i). Returns the AP holding the final
    h."""
    wpool, xtpool, hopool, pru_pool, pc_pool, pa_pool, scr_pool = pools
    (xw16, whr, whu, ch, gbr, gbu, cbc, ones, idt, idt16) = consts

    for q in range(chunk // 4):
        q0 = q * 4
        # transpose 4 x_t's: [BSH, D] -> [D, BSH] via PE (fp16), stage in SBUF
        xt_ps = scr_pool.tile([128, 4, 128], F16, tag="scr",
                              padded_shape=[128, 4, 256],
                              name=f"xtp_{tag}_{q}")
        for i in range(4):
            nc.tensor.transpose(xt_ps[:, i, :], xch[:, q0 + i, :], idt16[:])
        xt4 = xtpool.tile([D, 4, BSH], F16, tag="xt", name=f"xt_{tag}_{q}")
        nc.scalar.activation(xt4[:], xt_ps[:], AF.Copy)

        pr4 = pru_pool.tile([D, 4, BSH], F32, tag="pr4", name=f"pr4_{tag}_{q}")
        pu4 = pru_pool.tile([D, 4, BSH], F32, tag="pu4", name=f"pu4_{tag}_{q}")
        pc4 = pc_pool.tile([D, 4, BSH], F32, tag="pc4", name=f"pc4_{tag}_{q}")
        pa4 = pa_pool.tile([D, 4, BSH], F32, tag="pa4", name=f"pa4_{tag}_{q}")
        nc.tensor.matmul(pr4[:], xw16[:, 0, :], xt4[:], start=True, stop=True)
        nc.tensor.matmul(pu4[:], xw16[:, 1, :], xt4[:], start=True, stop=True)
        nc.tensor.matmul(pc4[:], xw16[:, 2, :], xt4[:], start=True, stop=True)
        nc.tensor.matmul(pa4[:], ones[:], ach[0:1, bass.ts(q, 4 * BSH)],
                         start=True, stop=True)

        ht_ps = scr_pool.tile([128, 4, 128], F32, tag="scr",
                              name=f"htp_{tag}_{q}")
        ho4 = hopool.tile([BSH, 4, D], F16, tag="ho", name=f"ho_{tag}_{q}")

        def emit_out(j, h_j):
            # output path for step j: PE transpose back to [BSH, D], then
            # mask on ACT (Copy with per-partition scale m_t); emitted one
            # step late so it lands in PE/ACT idle windows off the chain
            nc.tensor.transpose(ht_ps[:, j, :], h_j, idt[:])
            nc.scalar.activation(ho4[:, j, :], ht_ps[:, j, :], AF.Copy,
                                 scale=mch[:, q0 + j : q0 + j + 1])

        for i in range(4):
            h_c = h_cur
            # --- critical chain ---------------------------------------
            nc.tensor.matmul(pr4[:, i, :], whr[:], h_c,
                             start=False, stop=True, skip_group_check=True)
            r_t = wpool.tile([D, BSH], F32, tag="r", name=f"r_{tag}_{q}_{i}")
            nc.scalar.activation(r_t[:], pr4[:, i, :], AF.Sigmoid, bias=gbr[:])
            # u-path interleaved so in-order ACT does sigma_u in the gap
            nc.tensor.matmul(pu4[:, i, :], whu[:], h_c,
                             start=False, stop=True, skip_group_check=True)
            u_t = wpool.tile([D, BSH], F32, tag="u", name=f"u_{tag}_{q}_{i}")
            nc.scalar.activation(u_t[:], pu4[:, i, :], AF.Sigmoid, bias=gbu[:])
            rh = wpool.tile([D, BSH], F32, tag="rh", name=f"rh_{tag}_{q}_{i}")
            nc.vector.tensor_mul(rh[:], r_t[:], h_c)
            nc.tensor.matmul(pc4[:, i, :], ch[:], rh[:],
                             start=False, stop=True, skip_group_check=True)
            c_t = wpool.tile([D, BSH], F32, tag="c", name=f"c_{tag}_{q}_{i}")
            nc.scalar.activation(c_t[:], pc4[:, i, :], AF.Tanh, bias=cbc[:])
            # --- off-chain tail ---------------------------------------
            z = wpool.tile([D, BSH], F32, tag="z", name=f"z_{tag}_{q}_{i}")
            nc.vector.tensor_mul(z[:], u_t[:], pa4[:, i, :])
            p_t = wpool.tile([D, BSH], F32, tag="p", name=f"p_{tag}_{q}_{i}")
            nc.gpsimd.tensor_mul(p_t[:], z[:], h_c)
            # h' = z*h + (1-z)*c = p - (z-1)*c
            g_t = wpool.tile([D, BSH], F32, tag="g", name=f"g_{tag}_{q}_{i}")
            nc.vector.scalar_tensor_tensor(g_t[:], z[:], 1.0, c_t[:],
                                           OP.subtract, OP.mult)
            h_new = wpool.tile([D, BSH], F32, tag="h", name=f"h_{tag}_{q}_{i}")
            nc.vector.tensor_sub(h_new[:], p_t[:], g_t[:])
            if i > 0:
                emit_out(i - 1, h_prev)
            h_prev = h_new[:]
            h_cur = h_new[:]
        emit_out(3, h_prev)
        if dyn:
            nc.sync.dma_start(OUT[:, bass.ds(c0 + q0, 4), :], ho4[:])
        else:
            nc.sync.dma_start(OUT[:, c0 + q0 : c0 + q0 + 4, :], ho4[:])
    return h_cur


def _build(nc, t_steps, chunk, looped):
    nchunks = t_steps // chunk
    X = nc.dram_tensor("X", (BSH, t_steps, D), F16, kind="ExternalInput")
    A = nc.dram_tensor("A", (1, t_steps * BSH), F32, kind="ExternalInput")
    M = nc.dram_tensor("M", (BSH, t_steps), F32, kind="ExternalInput")
    GK = nc.dram_tensor("GK", (2 * D, 2 * D), F32, kind="ExternalInput")
    CK = nc.dram_tensor("CK", (2 * D, D), F32, kind="ExternalInput")
    GBR = nc.dram_tensor("GBR", (D, 1), F32, kind="ExternalInput")
    GBU = nc.dram_tensor("GBU", (D, 1), F32, kind="ExternalInput")
    CBC = nc.dram_tensor("CBC", (D, 1), F32, kind="ExternalInput")
    IDT = nc.dram_tensor("IDT", (128, 128), F32, kind="ExternalInput")
    XW = nc.dram_tensor("XW", (D, 3 * D), F16, kind="ExternalInput")
    OUT = nc.dram_tensor("OUT", (BSH, t_steps, D), F16, kind="ExternalOutput")

    with tile.TileContext(nc) as tc:
        with (
            tc.tile_pool(name="const", bufs=1) as constp,
            tc.tile_pool(name="xch", bufs=2) as xpool,
            tc.tile_pool(name="ach", bufs=2) as apool,
            tc.tile_pool(name="mch", bufs=2) as mpool,
            tc.tile_pool(name="xt", bufs=2) as xtpool,
            tc.tile_pool(name="work", bufs=3) as wpool,
            tc.tile_pool(name="ho", bufs=2) as hopool,
            tc.tile_pool(name="pru", bufs=2, space="PSUM") as pru_pool,
            tc.tile_pool(name="pc", bufs=2, space="PSUM") as pc_pool,
            tc.tile_pool(name="pa", bufs=1, space="PSUM") as pa_pool,
            tc.tile_pool(name="scr", bufs=1, space="PSUM") as scr_pool,
        ):
            pools = (wpool, xtpool, hopool, pru_pool, pc_pool, pa_pool,
                     scr_pool)
            xw16 = constp.tile([D, 3, D], F16, tag="xw16")
            whr = constp.tile([D, D], F32, tag="whr")
            whu = constp.tile([D, D], F32, tag="whu")
            ch = constp.tile([D, D], F32, tag="ch")
            gbr = constp.tile([D, 1], F32, tag="gbr")
            gbu = constp.tile([D, 1], F32, tag="gbu")
            cbc = constp.tile([D, 1], F32, tag="cbc")
            ones = constp.tile([1, D], F32, tag="ones")
            idt = constp.tile([128, 128], F32, tag="idt")
            idt16 = constp.tile([128, 128], F16, tag="idt16")
            consts = (xw16, whr, whu, ch, gbr, gbu, cbc, ones, idt, idt16)

            nc.sync.dma_start(xw16[:], XW[:])
            nc.sync.dma_start(whr[:], GK[D : 2 * D, 0:D])
            nc.sync.dma_start(whu[:], GK[D : 2 * D, D : 2 * D])
            nc.sync.dma_start(ch[:], CK[D : 2 * D, :])
            nc.sync.dma_start(gbr[:], GBR[:])
            nc.sync.dma_start(gbu[:], GBU[:])
            nc.sync.dma_start(cbc[:], CBC[:])
            nc.sync.dma_start(idt[:], IDT[:])
            nc.scalar.activation(idt16[:], idt[:], AF.Copy)
            nc.gpsimd.memset(ones[:], 1.0)

            hst = constp.tile([D, BSH], F32, tag="hst", name="h_state")
            nc.gpsimd.memset(hst[:], 0.0)
            if looped:
                # fixed-address state tile: each iteration starts and ends
                # with h in hst
                with tc.For_i(0, nchunks, 1) as ci:
                    c0 = ci * chunk
                    xch = xpool.tile([BSH, chunk, D], F16, tag="xch",
                                     name="xch")
                    nc.sync.dma_start(xch[:], X[:, bass.ds(c0, chunk), :])
                    ach = apool.tile([1, chunk * BSH], F32, tag="ach",
                                     name="ach")
                    nc.sync.dma_start(
                        ach[:], A[0:1, bass.ds(c0 * BSH, chunk * BSH)])
                    mch = mpool.tile([BSH, chunk], F32, tag="mch",
                                     name="mch")
                    nc.sync.dma_start(mch[:], M[:, bass.ds(c0, chunk)])
                    h_end = _emit_chunk(nc, pools, consts, hst[:], c0,
                                        xch, ach, mch, OUT, chunk,
                                        dyn=True, tag="L")
                    nc.vector.tensor_copy(hst[:], h_end)
            else:
                h_cur = hst[:]
                for ci in range(nchunks):
                    c0 = ci * chunk
                    xch = xpool.tile([BSH, chunk, D], F16, tag="xch",
                                     name=f"xch_{ci}")
                    nc.sync.dma_start(xch[:], X[:, c0 : c0 + chunk, :])
                    ach = apool.tile([1, chunk * BSH], F32, tag="ach",
                                     name=f"ach_{ci}")
                    nc.sync.dma_start(ach[:],
                                      A[0:1, c0 * BSH : (c0 + chunk) * BSH])
                    mch = mpool.tile([BSH, chunk], F32, tag="mch",
                                     name=f"mch_{ci}")
                    nc.sync.dma_start(mch[:], M[:, c0 : c0 + chunk])
                    h_cur = _emit_chunk(nc, pools, consts, h_cur, c0,
                                        xch, ach, mch, OUT, chunk,
                                        dyn=False, tag=str(ci))

    nc.finalize()
    return nc


def _get_runner(t_steps, looped):
    key = (t_steps, looped)
    if key in _runner_cache:
        return _runner_cache[key]

    import jax
    import jax.numpy as jnp
    from jax.sharding import Mesh, NamedSharding, PartitionSpec
    from jax.experimental.shard_map import shard_map
    from concourse.bass2jax import (
        _bass_exec_p,
        install_neuronx_cc_hook,
        partition_id_tensor,
    )

    install_neuronx_cc_hook()

    nc = bacc.Bacc("TRN2", target_bir_lowering=False)
    nc = _build(nc, t_steps, CHUNK, looped)
    assert nc.dbg_addr is None
    partition_name = (
        nc.partition_id_tensor.name if nc.partition_id_tensor else None
    )

    in_names, out_names, out_avals = [], [], []
    for alloc in nc.m.functions[0].allocations:
        if not isinstance(alloc, mybir.MemoryLocationSet):
            continue
        name = alloc.memorylocations[0].name
        if alloc.kind == "ExternalInput":
            if name != partition_name:
                in_names.append(name)
        elif alloc.kind == "ExternalOutput":
            assert alloc.tensor_shape is not None and alloc.dtype is not None
            out_names.append(name)
            out_avals.append(
                jax.core.ShapedArray(
                    tuple(alloc.tensor_shape), mybir.dt.np(alloc.dtype)
                )
            )
    n_params = len(in_names)
    n_outs = len(out_names)
    all_in_names = tuple(in_names) + tuple(out_names)
    if partition_name is not None:
        all_in_names = all_in_names + (partition_name,)

    devices = jax.devices()[:NCORES]
    assert len(devices) == NCORES
    mesh = Mesh(np.asarray(devices), ("core",))

    def _body(*args):
        operands = list(args)
        if partition_name is not None:
            operands.append(partition_id_tensor())
        outs = _bass_exec_p.bind(
            *operands,
            out_avals=tuple(out_avals),
            in_names=all_in_names,
            out_names=tuple(out_names),
            lowering_input_output_aliases=(),
            sim_require_finite=True,
            sim_require_nnan=True,
            nc=nc,
        )
        return tuple(outs)

    donate = tuple(range(n_params, n_params + n_outs))
    sharded = jax.jit(
        shard_map(
            _body,
            mesh=mesh,
            in_specs=(PartitionSpec("core"),) * (n_params + n_outs),
            out_specs=(PartitionSpec("core"),) * n_outs,
            check_rep=False,
        ),
        donate_argnums=donate,
        keep_unused=True,
    )

    out_sharding = NamedSharding(mesh, PartitionSpec("core"))
    zshape = (NCORES * out_avals[0].shape[0],) + tuple(out_avals[0].shape[1:])
    zdtype = jnp.dtype(out_avals[0].dtype)
    zeros_fn = jax.jit(
        lambda: jnp.zeros(zshape, zdtype), out_shardings=out_sharding
    )

    runner = (sharded, zeros_fn, list(in_names), list(devices), out_sharding)
    _runner_cache[key] = runner
    return runner


def kernel(rnn_input, att_score, gate_kernel, gate_bias, cand_kernel,
           cand_bias, sequence_length, _t_steps: int = T,
           _looped: bool = True):
    """Full-input entry point: shard across 8 cores, run, gather."""
    t_steps = _t_steps
    att = np.asarray(att_score, dtype=np.float32)
    gk = np.ascontiguousarray(np.asarray(gate_kernel, dtype=np.float32))
    gb = np.asarray(gate_bias, dtype=np.float32).reshape(2 * D)
    ck = np.ascontiguousarray(np.asarray(cand_kernel, dtype=np.float32))
    cb = np.asarray(cand_bias, dtype=np.float32).reshape(D)
    lens = np.asarray(sequence_length, dtype=np.int32).reshape(-1)

    from concourse._compat import axon_active

    fast = axon_active()
    if fast:
        import jax

        sharded, zeros_fn, in_names, devices, out_sharding = _get_runner(
            t_steps, _looped
        )
        zeros = zeros_fn()                   # async device-side zero-fill

        # per-shard h2d of X: convert shard c+1 to fp16 while shard c is
        # in flight over the transport
        xsrc = np.asarray(rnn_input)[:, :t_steps, :]
        xshards = [
            jax.device_put(
                np.ascontiguousarray(xsrc[c * BSH : (c + 1) * BSH])
                .astype(np.float16),
                devices[c],
            )
            for c in range(NCORES)
        ]
        Xg = jax.make_array_from_single_device_arrays(
            (B, t_steps, D), out_sharding, xshards
        )
    else:
        Xg = np.asarray(rnn_input)[:, :t_steps, :].astype(np.float16)

    alpha = 1.0 - att[:, :t_steps, 0]                       # [B, t]
    A = np.ascontiguousarray(
        alpha.reshape(NCORES, BSH, t_steps).transpose(0, 2, 1)
    ).reshape(NCORES, t_steps * BSH)
    M = (
        np.arange(t_steps, dtype=np.int32)[None, :]
        < np.minimum(lens, t_steps)[:, None]
    ).astype(np.float32)                                     # [B, t]

    arrays = {
        "X": Xg,
        "A": A,
        "M": M,
        "GK": np.tile(gk, (NCORES, 1)),
        "CK": np.tile(ck, (NCORES, 1)),
        "GBR": np.tile(gb[:D].reshape(D, 1), (NCORES, 1)),
        "GBU": np.tile(gb[D:].reshape(D, 1), (NCORES, 1)),
        "CBC": np.tile(cb.reshape(D, 1), (NCORES, 1)),
        "IDT": np.tile(np.eye(128, dtype=np.float32), (NCORES, 1)),
        "XW": np.tile(
            np.concatenate([gk[:D, :D], gk[:D, D:], ck[:D, :]], axis=1)
            .astype(np.float16),
            (NCORES, 1),
        ),
    }
    if not fast:
        # native (non-axon) path: same BIR via the stock SPMD runner
        from concourse.bass_utils import run_bass_kernel_spmd

        nc = _nc_cache.get((t_steps, _looped))
        if nc is None:
            nc = bacc.Bacc("TRN2", target_bir_lowering=False)
            nc = _build(nc, t_steps, CHUNK, _looped)
            _nc_cache[(t_steps, _looped)] = nc
        in_maps = []
        for c in range(NCORES):
            m = {k: v for k, v in arrays.items() if k not in ("X", "A", "M")}
            m["X"] = np.ascontiguousarray(Xg[c * BSH : (c + 1) * BSH])
            m["A"] = np.ascontiguousarray(arrays["A"][c : c + 1])
            m["M"] = np.ascontiguousarray(
                arrays["M"][c * BSH : (c + 1) * BSH]
            )
            for k in ("GK", "CK", "GBR", "GBU", "CBC", "IDT", "XW"):
                m[k] = np.ascontiguousarray(
                    arrays[k][: arrays[k].shape[0] // NCORES]
                )
            in_maps.append(m)
        res8 = run_bass_kernel_spmd(nc, in_maps, list(range(NCORES)))
        res = np.empty((B, t_steps, D), np.float32)
        for c in range(NCORES):
            res[c * BSH : (c + 1) * BSH] = res8.results[c]["OUT"]
        return res

    ins = [arrays[n] for n in in_names]
    outs = sharded(*ins, zeros)

    # per-shard d2h with async prefetch: upcast shard c to fp32 while
    # shard c+1 is in flight
    out = outs[0]
    shards = sorted(out.addressable_shards, key=lambda s: s.index[0].start)
    for s in shards:
        try:
            s.data.copy_to_host_async()
        except Exception:
            pass
    res = np.empty((B, t_steps, D), np.float32)
    for s in shards:
        res[s.index[0]] = np.asarray(s.data)
    return res
